# revision 5
# baseline (speedup 1.0000x reference)
"""ODE-VAE decoder TRN2 kernel (8-core data-parallel over batch).

Per-core layout: activations kept feature-major ("transposed", [feat, batch])
so every linear layer uses its weight matrix [fan_in, fan_out] directly as the
matmul stationary operand (out = lhsT.T @ rhs). Per-core batch = 512 = one
fp32 PSUM bank of moving-operand columns per matmul.

Dtypes: the ODE layer-1 / z-path / classifier matmuls run in float32r
(full-rate on the PE, ~1e-4 rel err); the ODE layer-2 matmul runs in bf16
(tanh output is in (-1,1), weights host-cast) to fit both weight matrices plus
state in SBUF. All accumulation and RK4 state updates are fp32.
"""
import sys

sys.path.insert(0, "/opt/trn_rl_repo")

import numpy as np
import ml_dtypes

import concourse.bass as bass
from concourse import bacc
import concourse.mybir as mybir
import concourse.tile as tile
from concourse.masks import make_identity
from concourse.bass_utils import run_bass_kernel_spmd

F32 = mybir.dt.float32
F32R = mybir.dt.float32r
BF16 = mybir.dt.bfloat16
AT = mybir.ActivationFunctionType
OP = mybir.AluOpType
AX = mybir.AxisListType

B, H, D, OH, C, T = 4096, 1024, 256, 2048, 128, 21
NCORES = 8
BS = B // NCORES          # 512 rows per core
HT, DT, OT = H // 128, D // 128, OH // 128   # 8, 2, 16 tiles
BT = BS // 128            # 4 batch chunks per core
N = BS                    # moving free dim per matmul

_CACHE = {}
TRACE = [False]
TRACE_KW = [{}]
LAST_RES = [None]


def _build(dts):
    nsteps = len(dts)
    nc = bacc.Bacc("TRN2", target_bir_lowering=False)

    h_d = nc.declare_dram_parameter("h", [BS, H], F32, isOutput=False)
    eps_d = nc.declare_dram_parameter("eps", [BS, D], F32, isOutput=False)
    fcmuW_d = nc.declare_dram_parameter("fc_mu_W", [H, D], F32R, isOutput=False)
    fcmub_d = nc.declare_dram_parameter("fc_mu_b", [D], F32, isOutput=False)
    fcvarW_d = nc.declare_dram_parameter("fc_var_W", [H, D], F32R, isOutput=False)
    fcvarb_d = nc.declare_dram_parameter("fc_var_b", [D], F32, isOutput=False)
    mapW_d = nc.declare_dram_parameter("map_W", [D, H], F32R, isOutput=False)
    mapb_d = nc.declare_dram_parameter("map_b", [H], F32, isOutput=False)
    odeW1_d = nc.declare_dram_parameter("ode_W1", [H, OH], F32R, isOutput=False)
    odeb1_d = nc.declare_dram_parameter("ode_b1", [OH], F32, isOutput=False)
    odeW2_d = nc.declare_dram_parameter("ode_W2b", [OH, H], BF16, isOutput=False)
    odeb2_d = nc.declare_dram_parameter("ode_b2", [H], F32, isOutput=False)
    finW_d = nc.declare_dram_parameter("fin_W", [H, C], F32R, isOutput=False)
    finb_d = nc.declare_dram_parameter("fin_b", [C], F32, isOutput=False)

    probs_d = nc.declare_dram_parameter("out_probs", [T, BS, C], F32, isOutput=True)
    mu_d = nc.declare_dram_parameter("out_mu", [BS, D], F32, isOutput=True)
    lv_d = nc.declare_dram_parameter("out_lv", [BS, D], F32, isOutput=True)

    with tile.TileContext(nc) as tc:
        with (
            tc.tile_pool(name="wpool", bufs=1) as wpool,
            tc.tile_pool(name="state", bufs=1) as state,
            tc.tile_pool(name="apool", bufs=3) as apool,
            tc.tile_pool(name="gpool", bufs=3) as gpool,
            tc.tile_pool(name="clsp", bufs=2) as clsp,
            tc.tile_pool(name="smallp", bufs=6) as smallp,
        ):
            ident = wpool.tile([128, 128], F32)
            make_identity(nc, ident)

            # ---- persistent small weights ----
            finW = wpool.tile([128, HT, C], F32R)
            b1t = wpool.tile([128, OT], F32)
            b2t = wpool.tile([128, HT], F32)
            finbt = wpool.tile([128, 1], F32)
            nc.sync.dma_start(out=finW, in_=finW_d.ap().rearrange("(kt p) m -> p kt m", p=128))
            nc.sync.dma_start(out=b1t, in_=odeb1_d.ap().rearrange("(m p) -> p m", p=128))
            nc.sync.dma_start(out=b2t, in_=odeb2_d.ap().rearrange("(m p) -> p m", p=128))
            nc.sync.dma_start(out=finbt, in_=finb_d.ap().rearrange("(m p) -> p m", p=128))

            # ---- state ----
            yA = state.tile([128, HT, N], F32R)
            yB = state.tile([128, HT, N], F32R)
            tbuf = state.tile([128, HT, N], F32R)
            ubuf = state.tile([128, OT, N], BF16)

            def classifier(y, t_idx):
                """softmax(y.T @ fin_W + fin_b) -> out_probs[t_idx]."""
                lp = pcls.tile([128, N], F32, tag="cp")
                for k in range(HT):
                    nc.tensor.matmul(lp, finW[:, k, :], y[:, k, :],
                                     start=(k == 0), stop=(k == HT - 1))
                logT = clsp.tile([128, N], F32, tag="logT")
                nc.vector.tensor_scalar(out=logT, in0=lp, scalar1=finbt[:, 0:1],
                                        scalar2=None, op0=OP.add)
                stage = clsp.tile([128, BT, C], F32, tag="stage")
                for bt in range(BT):
                    pt = pcls.tile([128, 128], F32, tag="cp")
                    nc.tensor.transpose(pt, logT[:, bt * 128:(bt + 1) * 128], ident)
                    nmx = smallp.tile([128, 1], F32, tag="nmx")
                    nc.vector.tensor_reduce(out=nmx, in_=pt, axis=AX.X, op=OP.max,
                                            negate=True)
                    e = clsp.tile([128, C], F32, tag="e")
                    nc.scalar.activation(out=e, in_=pt, func=AT.Exp, bias=nmx[:, 0:1])
                    s = smallp.tile([128, 1], F32, tag="s")
                    nc.vector.tensor_reduce(out=s, in_=e, axis=AX.X, op=OP.add)
                    r = smallp.tile([128, 1], F32, tag="r")
                    nc.vector.reciprocal(out=r, in_=s)
                    nc.vector.tensor_scalar(out=stage[:, bt, :], in0=e, scalar1=r[:, 0:1],
                                            scalar2=None, op0=OP.mult)
                nc.sync.dma_start(
                    out=probs_d.ap()[t_idx].rearrange("(bt p) c -> p bt c", p=128),
                    in_=stage)

            # ================= prologue =================
            with (
                tc.tile_pool(name="prol", bufs=1) as prol,
                tc.tile_pool(name="prodr", bufs=4) as prodr,
                tc.tile_pool(name="ppro", bufs=4, space="PSUM") as ppro,
            ):
                hl = prol.tile([128, BT, H], F32)
                epl = prol.tile([128, BT, D], F32)
                fcmuW = prol.tile([128, HT, D], F32R)
                fcvarW = prol.tile([128, HT, D], F32R)
                mapW = prol.tile([128, DT, H], F32R)
                mubc = prol.tile([128, D], F32)
                lvbc = prol.tile([128, D], F32)
                mapbt = prol.tile([128, HT], F32)
                nc.sync.dma_start(out=hl, in_=h_d.ap().rearrange("(bt p) m -> p bt m", p=128))
                nc.sync.dma_start(out=epl, in_=eps_d.ap().rearrange("(bt p) m -> p bt m", p=128))
                nc.sync.dma_start(out=fcmuW, in_=fcmuW_d.ap().rearrange("(kt p) m -> p kt m", p=128))
                nc.sync.dma_start(out=fcvarW, in_=fcvarW_d.ap().rearrange("(kt p) m -> p kt m", p=128))
                nc.sync.dma_start(out=mapW, in_=mapW_d.ap().rearrange("(kt p) m -> p kt m", p=128))
                nc.sync.dma_start(out=mubc, in_=fcmub_d.ap().unsqueeze(0).to_broadcast([128, D]))
                nc.sync.dma_start(out=lvbc, in_=fcvarb_d.ap().unsqueeze(0).to_broadcast([128, D]))
                nc.sync.dma_start(out=mapbt, in_=mapb_d.ap().rearrange("(m p) -> p m", p=128))

                # h^T via PE transposes
                hT = prol.tile([128, HT, N], F32R)
                for bt in range(BT):
                    for k in range(HT):
                        pt = ppro.tile([128, 128], F32, tag="pp")
                        nc.tensor.transpose(pt, hl[:, bt, k * 128:(k + 1) * 128], ident)
                        nc.scalar.copy(out=hT[:, k, bt * 128:(bt + 1) * 128], in_=pt)

                # mu / log_var (natural layout), std, zin
                mu_st = prol.tile([128, BT, D], F32)
                lv_st = prol.tile([128, BT, D], F32)
                zin_st = prol.tile([128, BT, D], F32)
                for bt in range(BT):
                    pmu = ppro.tile([128, D], F32, tag="pp")
                    for k in range(HT):
                        nc.tensor.matmul(pmu, hT[:, k, bt * 128:(bt + 1) * 128],
                                         fcmuW[:, k, :], start=(k == 0), stop=(k == HT - 1))
                    nc.vector.tensor_tensor(mu_st[:, bt, :], pmu, mubc, OP.add)
                    plv = ppro.tile([128, D], F32, tag="pp")
                    for k in range(HT):
                        nc.tensor.matmul(plv, hT[:, k, bt * 128:(bt + 1) * 128],
                                         fcvarW[:, k, :], start=(k == 0), stop=(k == HT - 1))
                    nc.vector.tensor_tensor(lv_st[:, bt, :], plv, lvbc, OP.add)
                    # zin = mu + exp(0.5*lv) * eps
                    std = prodr.tile([128, D], F32, tag="std")
                    nc.scalar.activation(out=std, in_=lv_st[:, bt, :], func=AT.Exp, scale=0.5)
                    se = prodr.tile([128, D], F32, tag="se")
                    nc.vector.tensor_tensor(se, std, epl[:, bt, :], OP.mult)
                    nc.vector.tensor_tensor(zin_st[:, bt, :], mu_st[:, bt, :], se, OP.add)
                nc.sync.dma_start(out=mu_d.ap().rearrange("(bt p) m -> p bt m", p=128), in_=mu_st)
                nc.sync.dma_start(out=lv_d.ap().rearrange("(bt p) m -> p bt m", p=128), in_=lv_st)

                # zin^T
                zinT = prol.tile([128, DT, N], F32R)
                for bt in range(BT):
                    for dch in range(DT):
                        pt = ppro.tile([128, 128], F32, tag="pp")
                        nc.tensor.transpose(pt, zin_st[:, bt, dch * 128:(dch + 1) * 128], ident)
                        nc.scalar.copy(out=zinT[:, dch, bt * 128:(bt + 1) * 128], in_=pt)

                # y0 = map_W.T @ zin^T + map_b
                for m in range(HT):
                    pz = ppro.tile([128, N], F32, tag="pz")
                    for k in range(DT):
                        nc.tensor.matmul(pz, mapW[:, k, m * 128:(m + 1) * 128],
                                         zinT[:, k, :], start=(k == 0), stop=(k == DT - 1))
                    nc.vector.tensor_scalar(out=yA[:, m, :], in0=pz,
                                            scalar1=mapbt[:, m:m + 1], scalar2=None,
                                            op0=OP.add)

            bigw = tc.tile_pool(name="bigw", bufs=1)
            big = bigw.__enter__()
            pmm_ctx = tc.tile_pool(name="pmm", bufs=6, space="PSUM")
            pmm = pmm_ctx.__enter__()
            pcls_ctx = tc.tile_pool(name="pcls", bufs=2, space="PSUM")
            pcls = pcls_ctx.__enter__()
            W1 = big.tile([128, HT, OH], F32R)
            W2 = big.tile([128, OT, H], BF16)
            nc.sync.dma_start(out=W1, in_=odeW1_d.ap().rearrange("(kt p) m -> p kt m", p=128))
            nc.sync.dma_start(out=W2, in_=odeW2_d.ap().rearrange("(kt p) m -> p kt m", p=128))

            classifier(yA, 0)

            # ================= RK4 steps =================
            def feval(rhs, j, dt, y, yn):
                """One f-eval: k_j = f(rhs); update tbuf / yn. m-outer order:
                each PSUM tile finishes early and drains while the PE streams
                the next m-tile, so 2-3 banks suffice and drains never burst."""
                # layer 1: u = tanh(rhs @ W1 + b1)
                for m in range(OT):
                    p = pmm.tile([128, N], F32, tag="pm", name=f"pm1_{m}")
                    for kk in range(HT):
                        nc.tensor.matmul(p, W1[:, kk, m * 128:(m + 1) * 128],
                                         rhs[:, kk, :], start=(kk == 0),
                                         stop=(kk == HT - 1))
                    nc.scalar.activation(out=ubuf[:, m, :], in_=p,
                                         func=AT.Tanh, bias=b1t[:, m:m + 1])
                # layer 2: psum = u @ W2
                for mo in range(HT):
                    p = pmm.tile([128, N], F32, tag="pm", name=f"pm2_{mo}")
                    for kk in range(OT):
                        nc.tensor.matmul(p, W2[:, kk, mo * 128:(mo + 1) * 128],
                                         ubuf[:, kk, :], start=(kk == 0),
                                         stop=(kk == OT - 1))
                    b2ap = b2t[:, mo:mo + 1]
                    yv = y[:, mo, :].bitcast(F32)
                    if j < 4:
                        ac = (dt / 2.0, dt / 2.0, dt)[j - 1]
                        a = apool.tile([128, N], F32, tag="a")
                        nc.vector.tensor_scalar(out=a, in0=p, scalar1=b2ap,
                                                scalar2=ac, op0=OP.add, op1=OP.mult)
                        nc.vector.tensor_tensor(tbuf[:, mo, :], yv, a, OP.add)
                    gc = (dt / 6.0, dt / 3.0, dt / 3.0, dt / 6.0)[j - 1]
                    g = gpool.tile([128, N], F32, tag="g")
                    nc.vector.tensor_scalar(out=g, in0=p, scalar1=b2ap,
                                            scalar2=gc, op0=OP.add, op1=OP.mult)
                    if j == 1:
                        nc.vector.tensor_tensor(yn[:, mo, :], yv, g, OP.add)
                    else:
                        nc.vector.tensor_tensor(yn[:, mo, :], yn[:, mo, :].bitcast(F32),
                                                g, OP.add)

            for n in range(nsteps):
                dt = float(dts[n])
                y, yn = (yA, yB) if n % 2 == 0 else (yB, yA)
                with nc.named_scope(f"step{n}"):
                    feval(y, 1, dt, y, yn)
                    feval(tbuf, 2, dt, y, yn)
                    feval(tbuf, 3, dt, y, yn)
                    feval(tbuf, 4, dt, y, yn)
                classifier(yn, n + 1)

            pcls_ctx.__exit__(None, None, None)
            pmm_ctx.__exit__(None, None, None)
            bigw.__exit__(None, None, None)

    nc.compile()
    return nc


def kernel(h, eps, timestamps, fc_mu_W, fc_mu_b, fc_var_W, fc_var_b,
           map_W, map_b, ode_W1, ode_b1, ode_W2, ode_b2, fin_W, fin_b):
    h = np.asarray(h, dtype=np.float32)
    eps = np.asarray(eps, dtype=np.float32)
    timestamps = np.asarray(timestamps, dtype=np.float32)
    dts = np.diff(timestamps)
    key = tuple(np.asarray(dts).tolist())
    if key not in _CACHE:
        _CACHE[key] = _build(dts)
    nc = _CACHE[key]

    shared = dict(
        fc_mu_W=np.asarray(fc_mu_W, np.float32), fc_mu_b=np.asarray(fc_mu_b, np.float32),
        fc_var_W=np.asarray(fc_var_W, np.float32), fc_var_b=np.asarray(fc_var_b, np.float32),
        map_W=np.asarray(map_W, np.float32), map_b=np.asarray(map_b, np.float32),
        ode_W1=np.asarray(ode_W1, np.float32), ode_b1=np.asarray(ode_b1, np.float32),
        ode_W2b=np.asarray(ode_W2, np.float32).astype(ml_dtypes.bfloat16),
        ode_b2=np.asarray(ode_b2, np.float32),
        fin_W=np.asarray(fin_W, np.float32), fin_b=np.asarray(fin_b, np.float32),
    )
    in_maps = []
    for c in range(NCORES):
        m = dict(shared)
        m["h"] = h[c * BS:(c + 1) * BS]
        m["eps"] = eps[c * BS:(c + 1) * BS]
        in_maps.append(m)

    res = run_bass_kernel_spmd(nc, in_maps, list(range(NCORES)),
                               trace=TRACE[0], **TRACE_KW[0])
    LAST_RES[0] = res
    probs = np.concatenate([r["out_probs"] for r in res.results], axis=1)
    mu = np.concatenate([r["out_mu"] for r in res.results], axis=0)
    lv = np.concatenate([r["out_lv"] for r in res.results], axis=0)
    return probs, mu, lv


# revision 6
# speedup vs baseline: 1.0891x; 1.0891x over previous
"""ODE-VAE decoder TRN2 kernel (8-core data-parallel over batch).

Per-core layout: activations kept feature-major ("transposed", [feat, batch])
so every linear layer uses its weight matrix [fan_in, fan_out] directly as the
matmul stationary operand (out = lhsT.T @ rhs). Per-core batch = 512 = one
fp32 PSUM bank of moving-operand columns per matmul.

Dtypes: the ODE layer-1 / z-path / classifier matmuls run in float32r
(full-rate on the PE, ~1e-4 rel err); the ODE layer-2 matmul runs in bf16
(tanh output is in (-1,1), weights host-cast) to fit both weight matrices plus
state in SBUF. All accumulation and RK4 state updates are fp32.
"""
import sys

sys.path.insert(0, "/opt/trn_rl_repo")

import numpy as np
import ml_dtypes

import concourse.bass as bass
from concourse import bacc
import concourse.mybir as mybir
import concourse.tile as tile
from concourse.masks import make_identity
from concourse.bass_utils import run_bass_kernel_spmd

F32 = mybir.dt.float32
F32R = mybir.dt.float32r
BF16 = mybir.dt.bfloat16
AT = mybir.ActivationFunctionType
OP = mybir.AluOpType
AX = mybir.AxisListType

B, H, D, OH, C, T = 4096, 1024, 256, 2048, 128, 21
NCORES = 8
BS = B // NCORES          # 512 rows per core
HT, DT, OT = H // 128, D // 128, OH // 128   # 8, 2, 16 tiles
BT = BS // 128            # 4 batch chunks per core
N = BS                    # moving free dim per matmul

_CACHE = {}
TRACE = [False]
TRACE_KW = [{}]
LAST_RES = [None]


def _build(dts):
    nsteps = len(dts)
    nc = bacc.Bacc("TRN2", target_bir_lowering=False)

    h_d = nc.declare_dram_parameter("h", [BS, H], F32, isOutput=False)
    eps_d = nc.declare_dram_parameter("eps", [BS, D], F32, isOutput=False)
    fcmuW_d = nc.declare_dram_parameter("fc_mu_W", [H, D], F32R, isOutput=False)
    fcmub_d = nc.declare_dram_parameter("fc_mu_b", [D], F32, isOutput=False)
    fcvarW_d = nc.declare_dram_parameter("fc_var_W", [H, D], F32R, isOutput=False)
    fcvarb_d = nc.declare_dram_parameter("fc_var_b", [D], F32, isOutput=False)
    mapW_d = nc.declare_dram_parameter("map_W", [D, H], F32R, isOutput=False)
    mapb_d = nc.declare_dram_parameter("map_b", [H], F32, isOutput=False)
    odeW1_d = nc.declare_dram_parameter("ode_W1", [H, OH], F32R, isOutput=False)
    odeb1_d = nc.declare_dram_parameter("ode_b1", [OH], F32, isOutput=False)
    odeW2_d = nc.declare_dram_parameter("ode_W2b", [OH, H], BF16, isOutput=False)
    odeb2_d = nc.declare_dram_parameter("ode_b2", [H], F32, isOutput=False)
    finW_d = nc.declare_dram_parameter("fin_W", [H, C], F32R, isOutput=False)
    finb_d = nc.declare_dram_parameter("fin_b", [C], F32, isOutput=False)

    probs_d = nc.declare_dram_parameter("out_probs", [T, BS, C], F32, isOutput=True)
    mu_d = nc.declare_dram_parameter("out_mu", [BS, D], F32, isOutput=True)
    lv_d = nc.declare_dram_parameter("out_lv", [BS, D], F32, isOutput=True)

    with tile.TileContext(nc) as tc:
        with (
            tc.tile_pool(name="wpool", bufs=1) as wpool,
            tc.tile_pool(name="state", bufs=1) as state,
            tc.tile_pool(name="apool", bufs=3) as apool,
            tc.tile_pool(name="gpool", bufs=3) as gpool,
            tc.tile_pool(name="clsp", bufs=2) as clsp,
            tc.tile_pool(name="smallp", bufs=6) as smallp,
        ):
            ident = wpool.tile([128, 128], F32)
            make_identity(nc, ident)

            # ---- persistent small weights ----
            finW = wpool.tile([128, HT, C], F32R)
            b1t = wpool.tile([128, OT], F32)
            b2t = wpool.tile([128, HT], F32)
            finbt = wpool.tile([128, 1], F32)
            nc.sync.dma_start(out=finW, in_=finW_d.ap().rearrange("(kt p) m -> p kt m", p=128))
            nc.sync.dma_start(out=b1t, in_=odeb1_d.ap().rearrange("(m p) -> p m", p=128))
            nc.sync.dma_start(out=b2t, in_=odeb2_d.ap().rearrange("(m p) -> p m", p=128))
            nc.sync.dma_start(out=finbt, in_=finb_d.ap().rearrange("(m p) -> p m", p=128))

            # ---- state ----
            yA = state.tile([128, HT, N], F32R)
            yB = state.tile([128, HT, N], F32R)
            tbuf = state.tile([128, HT, N], F32R)
            ubuf = state.tile([128, OT, N], BF16)

            def classifier(y, t_idx):
                """softmax(y.T @ fin_W + fin_b) -> out_probs[t_idx]."""
                lp = pcls.tile([128, N], F32, tag="cp")
                for k in range(HT):
                    nc.tensor.matmul(lp, finW[:, k, :], y[:, k, :],
                                     start=(k == 0), stop=(k == HT - 1))
                logT = clsp.tile([128, N], F32, tag="logT")
                nc.vector.tensor_scalar(out=logT, in0=lp, scalar1=finbt[:, 0:1],
                                        scalar2=None, op0=OP.add)
                stage = clsp.tile([128, BT, C], F32, tag="stage")
                for bt in range(BT):
                    pt = pcls.tile([128, 128], F32, tag="cp")
                    nc.tensor.transpose(pt, logT[:, bt * 128:(bt + 1) * 128], ident)
                    nmx = smallp.tile([128, 1], F32, tag="nmx")
                    nc.vector.tensor_reduce(out=nmx, in_=pt, axis=AX.X, op=OP.max,
                                            negate=True)
                    e = clsp.tile([128, C], F32, tag="e")
                    nc.scalar.activation(out=e, in_=pt, func=AT.Exp, bias=nmx[:, 0:1])
                    s = smallp.tile([128, 1], F32, tag="s")
                    nc.vector.tensor_reduce(out=s, in_=e, axis=AX.X, op=OP.add)
                    r = smallp.tile([128, 1], F32, tag="r")
                    nc.vector.reciprocal(out=r, in_=s)
                    nc.vector.tensor_scalar(out=stage[:, bt, :], in0=e, scalar1=r[:, 0:1],
                                            scalar2=None, op0=OP.mult)
                nc.sync.dma_start(
                    out=probs_d.ap()[t_idx].rearrange("(bt p) c -> p bt c", p=128),
                    in_=stage)

            # ================= prologue =================
            with (
                tc.tile_pool(name="prol", bufs=1) as prol,
                tc.tile_pool(name="prodr", bufs=4) as prodr,
                tc.tile_pool(name="ppro", bufs=4, space="PSUM") as ppro,
            ):
                hl = prol.tile([128, BT, H], F32)
                epl = prol.tile([128, BT, D], F32)
                fcmuW = prol.tile([128, HT, D], F32R)
                fcvarW = prol.tile([128, HT, D], F32R)
                mapW = prol.tile([128, DT, H], F32R)
                mubc = prol.tile([128, D], F32)
                lvbc = prol.tile([128, D], F32)
                mapbt = prol.tile([128, HT], F32)
                nc.sync.dma_start(out=hl, in_=h_d.ap().rearrange("(bt p) m -> p bt m", p=128))
                nc.sync.dma_start(out=epl, in_=eps_d.ap().rearrange("(bt p) m -> p bt m", p=128))
                nc.sync.dma_start(out=fcmuW, in_=fcmuW_d.ap().rearrange("(kt p) m -> p kt m", p=128))
                nc.sync.dma_start(out=fcvarW, in_=fcvarW_d.ap().rearrange("(kt p) m -> p kt m", p=128))
                nc.sync.dma_start(out=mapW, in_=mapW_d.ap().rearrange("(kt p) m -> p kt m", p=128))
                nc.sync.dma_start(out=mubc, in_=fcmub_d.ap().unsqueeze(0).to_broadcast([128, D]))
                nc.sync.dma_start(out=lvbc, in_=fcvarb_d.ap().unsqueeze(0).to_broadcast([128, D]))
                nc.sync.dma_start(out=mapbt, in_=mapb_d.ap().rearrange("(m p) -> p m", p=128))

                # h^T via PE transposes
                hT = prol.tile([128, HT, N], F32R)
                for bt in range(BT):
                    for k in range(HT):
                        pt = ppro.tile([128, 128], F32, tag="pp")
                        nc.tensor.transpose(pt, hl[:, bt, k * 128:(k + 1) * 128], ident)
                        nc.scalar.copy(out=hT[:, k, bt * 128:(bt + 1) * 128], in_=pt)

                # mu / log_var (natural layout), std, zin
                mu_st = prol.tile([128, BT, D], F32)
                lv_st = prol.tile([128, BT, D], F32)
                zin_st = prol.tile([128, BT, D], F32)
                for bt in range(BT):
                    pmu = ppro.tile([128, D], F32, tag="pp")
                    for k in range(HT):
                        nc.tensor.matmul(pmu, hT[:, k, bt * 128:(bt + 1) * 128],
                                         fcmuW[:, k, :], start=(k == 0), stop=(k == HT - 1))
                    nc.vector.tensor_tensor(mu_st[:, bt, :], pmu, mubc, OP.add)
                    plv = ppro.tile([128, D], F32, tag="pp")
                    for k in range(HT):
                        nc.tensor.matmul(plv, hT[:, k, bt * 128:(bt + 1) * 128],
                                         fcvarW[:, k, :], start=(k == 0), stop=(k == HT - 1))
                    nc.vector.tensor_tensor(lv_st[:, bt, :], plv, lvbc, OP.add)
                    # zin = mu + exp(0.5*lv) * eps
                    std = prodr.tile([128, D], F32, tag="std")
                    nc.scalar.activation(out=std, in_=lv_st[:, bt, :], func=AT.Exp, scale=0.5)
                    se = prodr.tile([128, D], F32, tag="se")
                    nc.vector.tensor_tensor(se, std, epl[:, bt, :], OP.mult)
                    nc.vector.tensor_tensor(zin_st[:, bt, :], mu_st[:, bt, :], se, OP.add)
                nc.sync.dma_start(out=mu_d.ap().rearrange("(bt p) m -> p bt m", p=128), in_=mu_st)
                nc.sync.dma_start(out=lv_d.ap().rearrange("(bt p) m -> p bt m", p=128), in_=lv_st)

                # zin^T
                zinT = prol.tile([128, DT, N], F32R)
                for bt in range(BT):
                    for dch in range(DT):
                        pt = ppro.tile([128, 128], F32, tag="pp")
                        nc.tensor.transpose(pt, zin_st[:, bt, dch * 128:(dch + 1) * 128], ident)
                        nc.scalar.copy(out=zinT[:, dch, bt * 128:(bt + 1) * 128], in_=pt)

                # y0 = map_W.T @ zin^T + map_b
                for m in range(HT):
                    pz = ppro.tile([128, N], F32, tag="pz")
                    for k in range(DT):
                        nc.tensor.matmul(pz, mapW[:, k, m * 128:(m + 1) * 128],
                                         zinT[:, k, :], start=(k == 0), stop=(k == DT - 1))
                    nc.vector.tensor_scalar(out=yA[:, m, :], in0=pz,
                                            scalar1=mapbt[:, m:m + 1], scalar2=None,
                                            op0=OP.add)

            bigw = tc.tile_pool(name="bigw", bufs=1)
            big = bigw.__enter__()
            pmm_ctx = tc.tile_pool(name="pmm", bufs=6, space="PSUM")
            pmm = pmm_ctx.__enter__()
            pcls_ctx = tc.tile_pool(name="pcls", bufs=2, space="PSUM")
            pcls = pcls_ctx.__enter__()
            W1 = big.tile([128, HT, OH], F32R)
            W2 = big.tile([128, OT, H], BF16)
            nc.sync.dma_start(out=W1, in_=odeW1_d.ap().rearrange("(kt p) m -> p kt m", p=128))
            nc.sync.dma_start(out=W2, in_=odeW2_d.ap().rearrange("(kt p) m -> p kt m", p=128))

            classifier(yA, 0)

            # ================= RK4 steps =================
            def feval(rhs, j, dt, y, yn):
                """One f-eval: k_j = f(rhs); update tbuf / yn. m-outer order:
                each PSUM tile finishes early and drains while the PE streams
                the next m-tile, so 2-3 banks suffice and drains never burst."""
                # layer 1: u = tanh(rhs @ W1 + b1)
                for m in range(OT):
                    p = pmm.tile([128, N], F32, tag="pm", name=f"pm1_{m}")
                    for kk in range(HT):
                        nc.tensor.matmul(p, W1[:, kk, m * 128:(m + 1) * 128],
                                         rhs[:, kk, :], start=(kk == 0),
                                         stop=(kk == HT - 1))
                    nc.scalar.activation(out=ubuf[:, m, :], in_=p,
                                         func=AT.Tanh, bias=b1t[:, m:m + 1])
                # layer 2: psum = u @ W2
                for mo in range(HT):
                    p = pmm.tile([128, N], F32, tag="pm", name=f"pm2_{mo}")
                    for kk in range(OT):
                        nc.tensor.matmul(p, W2[:, kk, mo * 128:(mo + 1) * 128],
                                         ubuf[:, kk, :], start=(kk == 0),
                                         stop=(kk == OT - 1))
                    b2ap = b2t[:, mo:mo + 1]
                    yv = y[:, mo, :].bitcast(F32)
                    if j < 4:
                        ac = (dt / 2.0, dt / 2.0, dt)[j - 1]
                        a = apool.tile([128, N], F32, tag="a")
                        nc.vector.tensor_scalar(out=a, in0=p, scalar1=b2ap,
                                                scalar2=ac, op0=OP.add, op1=OP.mult)
                        nc.vector.tensor_tensor(tbuf[:, mo, :], yv, a, OP.add)
                    gc = (dt / 6.0, dt / 3.0, dt / 3.0, dt / 6.0)[j - 1]
                    g = gpool.tile([128, N], F32, tag="g")
                    nc.vector.tensor_scalar(out=g, in0=p, scalar1=b2ap,
                                            scalar2=gc, op0=OP.add, op1=OP.mult)
                    if j == 1:
                        nc.vector.tensor_tensor(yn[:, mo, :], yv, g, OP.add)
                    else:
                        nc.vector.tensor_tensor(yn[:, mo, :], yn[:, mo, :].bitcast(F32),
                                                g, OP.add)

            for n in range(nsteps):
                dt = float(dts[n])
                y, yn = (yA, yB) if n % 2 == 0 else (yB, yA)
                with nc.named_scope(f"step{n}"):
                    feval(y, 1, dt, y, yn)
                    feval(tbuf, 2, dt, y, yn)
                    feval(tbuf, 3, dt, y, yn)
                    feval(tbuf, 4, dt, y, yn)
                classifier(yn, n + 1)

            pcls_ctx.__exit__(None, None, None)
            pmm_ctx.__exit__(None, None, None)
            bigw.__exit__(None, None, None)

    nc.compile()
    return nc


def kernel(h, eps, timestamps, fc_mu_W, fc_mu_b, fc_var_W, fc_var_b,
           map_W, map_b, ode_W1, ode_b1, ode_W2, ode_b2, fin_W, fin_b):
    h = np.asarray(h, dtype=np.float32)
    eps = np.asarray(eps, dtype=np.float32)
    timestamps = np.asarray(timestamps, dtype=np.float32)
    dts = np.diff(timestamps)
    key = tuple(np.asarray(dts).tolist())
    if key not in _CACHE:
        _CACHE[key] = _build(dts)
    nc = _CACHE[key]

    shared = dict(
        fc_mu_W=np.asarray(fc_mu_W, np.float32), fc_mu_b=np.asarray(fc_mu_b, np.float32),
        fc_var_W=np.asarray(fc_var_W, np.float32), fc_var_b=np.asarray(fc_var_b, np.float32),
        map_W=np.asarray(map_W, np.float32), map_b=np.asarray(map_b, np.float32),
        ode_W1=np.asarray(ode_W1, np.float32), ode_b1=np.asarray(ode_b1, np.float32),
        ode_W2b=np.asarray(ode_W2, np.float32).astype(ml_dtypes.bfloat16),
        ode_b2=np.asarray(ode_b2, np.float32),
        fin_W=np.asarray(fin_W, np.float32), fin_b=np.asarray(fin_b, np.float32),
    )
    in_maps = []
    for c in range(NCORES):
        m = dict(shared)
        m["h"] = h[c * BS:(c + 1) * BS]
        m["eps"] = eps[c * BS:(c + 1) * BS]
        in_maps.append(m)

    import os
    if not TRACE[0]:
        # A stray BASS_TRACE in the environment would route through the NTFF
        # profiling path, which needs hooks this image may not have.
        os.environ["BASS_NEVER_TRACE"] = "1"
    else:
        os.environ.pop("BASS_NEVER_TRACE", None)
    res = run_bass_kernel_spmd(nc, in_maps, list(range(NCORES)),
                               trace=TRACE[0], **TRACE_KW[0])
    LAST_RES[0] = res
    probs = np.concatenate([r["out_probs"] for r in res.results], axis=1)
    mu = np.concatenate([r["out_mu"] for r in res.results], axis=0)
    lv = np.concatenate([r["out_lv"] for r in res.results], axis=0)
    return probs, mu, lv
